# revision 1
# baseline (speedup 1.0000x reference)
"""3-layer GAT on 8 Trainium2 NeuronCores (graph/data parallel by dst node).

See bottom `kernel()` for the entry point. Self-contained: only needs the
concourse/bass stack at /opt/trn_rl_repo and 8 axon-tunneled NeuronCores.

Design:
  - Nodes padded to 50176 = 8 cores x 49 blocks x 128. Core c owns dst nodes
    [6272c, 6272(c+1)); edges (+N self-loops) are bucketed by (core, block).
  - Per layer a DRAM gather TABLE holds one 512B row per node:
    [h bf16 x128 | ones bf16 x4 | a_src f32 x4 | a_dst f32 x4 | pad].
    Row 0 is an all-zero poison row (node n -> row n+1); padding edges point
    at zero rows so they contribute exactly 0.
  - Edge phase, per dst block: dma_gather the block's source rows (int16
    indices; lo/hi table-half split keeps every index < 32768; 4 rotating
    SWDGE queues ~2.5ns/row), w = exp(leaky_relu(a_src + a_dst)) on DVE+ACT,
    one-hot(dst_rel) by DVE is_equal, then one matmul per 128-edge chunk
    accumulates [slot, h*w | w] into PSUM. Epilogue: divide by the summed w
    (softmax without max-subtraction), +bias, leaky_relu, PE-transpose, and
    the NEXT layer's dense matmul x @ [W | v_src | v_dst] writes the next
    table slab. (v_src = W-folded att_src so a_src comes free with h.)
  - 4 SPMD launches: dense0 / edge0+dense1 / edge1+dense2 / edge2. The host
    only reassembles the full table from per-core slabs and re-indexes the
    per-edge a_dst stream between launches.
"""

import os
import sys
import copy
import types
import numpy as np

if "/opt/trn_rl_repo" not in sys.path:
    sys.path.insert(0, "/opt/trn_rl_repo")

N, E = 50000, 800000
NEG = 0.2

NCORES = 8
BLOCKS = 49                    # per core
NPC = BLOCKS * 128             # nodes per core = 6272
NPAD = NCORES * NPC            # 50176
TROWS = 50432                  # table rows: 0 = poison, node n -> n+1
LO_LIM = 32768                 # lo half = rows [0, 32768)
HI_OFF = 17664                 # hi half = rows [17664, 50432)
HI_POISON = 50400 - HI_OFF     # all-zero junk row inside hi half
ROWF = 128                     # f32 words per table row (512B)


# --------------------------------------------------------------------------
# harness shims
# --------------------------------------------------------------------------
def _install_ntff_hook():
    """Register the NTFF profile hook the agent image's antenv lacks, so
    run_bass_kernel_spmd(trace=True) can report exec_time_ns."""
    try:
        import antenv
        if getattr(antenv, "axon_hooks", None) is not None:
            return True
        mod = types.ModuleType("antenv.axon_hooks")
        hook = [None]
        mod.set_axon_ntff_profile_hook = lambda h: hook.__setitem__(0, h)
        mod.get_axon_ntff_profile_hook = lambda: hook[0]
        antenv.axon_hooks = mod
        sys.modules["antenv.axon_hooks"] = mod
        from trn_agent_boot.trn_boot import _ntff_profile_via_ctypes
        mod.set_axon_ntff_profile_hook(
            _ntff_profile_via_ctypes("/opt/axon/libaxon_pjrt.so"))
        return hook[0] is not None
    except Exception:
        return False


def _split_multiwait_ctrl(nc, max_waits=1):
    """This walrus build rejects >1 semaphore wait on CTRL-class (Drain/Nop)
    instructions; split the TileContext tail drain into single-wait clones."""
    for bb in nc.main_func.blocks:
        newlist = []
        for ins in bb.instructions:
            si = ins.sync_info
            if (si is not None and si.on_wait and len(si.on_wait) > max_waits
                    and type(ins).__name__ in ("InstDrain", "InstNop")):
                waits = list(si.on_wait)
                si.on_wait = type(si.on_wait)([waits[0]])
                for i, w in enumerate(waits[1:]):
                    cl = copy.deepcopy(ins)
                    cl.name = f"{ins.name}-wsplit{i}"
                    cl.sync_info = copy.deepcopy(si)
                    cl.sync_info.on_wait = type(si.on_wait)([w])
                    cl.sync_info.on_update = type(si.on_update)([])
                    nc.register_instruction(cl, overwrite=True)
                    newlist.append(cl)
            newlist.append(ins)
        bb.instructions[:] = newlist


# --------------------------------------------------------------------------
# host-side graph prep (static per graph, layer-independent)
# --------------------------------------------------------------------------
def _split_calls(k):
    out = []
    while k > 0:
        c = min(k, 8)          # dma_gather per-call limit: 1024 = 8 chunks
        out.append(c)
        k -= c
    return out


def _wrap_idx(idx):
    """[ni] -> [128, ni//16] int16 in dma_gather's 16-partition wrapped
    layout, replicated to all 8 GPSIMD cores."""
    ni = idx.shape[0]
    w = np.zeros((16, ni // 16), dtype=np.int16)
    w[np.arange(ni) % 16, np.arange(ni) // 16] = idx
    return np.tile(w, (8, 1))


def _prep_graph(edge_index):
    src = np.concatenate([np.asarray(edge_index[0], np.int64),
                          np.arange(N, dtype=np.int64)])
    dst = np.concatenate([np.asarray(edge_index[1], np.int64),
                          np.arange(N, dtype=np.int64)])
    core = dst // NPC
    blk = (dst % NPC) // 128
    half = (src > (LO_LIM - 2)).astype(np.int64)   # 0 = lo, 1 = hi
    key = (core * BLOCKS + blk) * 2 + half
    order = np.argsort(key, kind="stable")
    ks = key[order]
    bounds = np.searchsorted(ks, np.arange(NCORES * BLOCKS * 2 + 1))

    counts = np.diff(bounds).reshape(NCORES, BLOCKS, 2)
    klo = int(np.max((counts[:, :, 0] + 127) // 128))
    khi = int(np.max((counts[:, :, 1] + 127) // 128))
    K = klo + khi

    per_core = []
    for c in range(NCORES):
        gidx_parts = []
        dstrel = np.zeros((128, BLOCKS * K), dtype=np.float32)
        dstmap = np.full((BLOCKS, K, 128), -1, dtype=np.int64)
        for b in range(BLOCKS):
            for gi, kcnt in ((0, klo), (1, khi)):
                g = (c * BLOCKS + b) * 2 + gi
                es = order[bounds[g]:bounds[g + 1]]
                npadded = kcnt * 128
                idx = np.full(npadded, 0 if gi == 0 else HI_POISON, np.int64)
                dd = np.full(npadded, -1, np.int64)
                ne = len(es)
                if ne:
                    s = src[es]
                    idx[:ne] = (s + 1) if gi == 0 else (s + 1 - HI_OFF)
                    dd[:ne] = dst[es]
                kbase = 0 if gi == 0 else klo
                dm = dd.reshape(kcnt, 128)
                dstmap[b, kbase:kbase + kcnt] = dm
                dr = (dm % 128).astype(np.float32)
                dr[dm < 0] = 0
                dstrel[:, b * K + kbase:b * K + kbase + kcnt] = dr.T
                off = 0
                for nch in _split_calls(kcnt):
                    gidx_parts.append(_wrap_idx(idx[off:off + nch * 128]))
                    off += nch * 128
        per_core.append(dict(gidx=np.ascontiguousarray(
            np.concatenate(gidx_parts, axis=1)), dstrel=dstrel, dstmap=dstmap))
    calls = ([(0, n) for n in _split_calls(klo)]
             + [(1, n) for n in _split_calls(khi)])
    return dict(klo=klo, khi=khi, K=K, calls=calls, per_core=per_core)


def _wext(W, a_s, a_d):
    """[128, 136] = [W | v_src | v_dst] zero-padded; v_* = W @ att_* per head
    so a_src/a_dst fall out of the same dense matmul as h."""
    W = np.asarray(W, np.float32)
    a_s = np.asarray(a_s, np.float32)
    a_d = np.asarray(a_d, np.float32)
    heads, ch = a_s.shape
    out = np.zeros((128, 136), np.float32)
    out[:W.shape[0], :W.shape[1]] = W
    for h in range(heads):
        out[:W.shape[0], 128 + h] = W[:, h * ch:(h + 1) * ch] @ a_s[h]
        out[:W.shape[0], 132 + h] = W[:, h * ch:(h + 1) * ch] @ a_d[h]
    return out


def _assemble_table(slabs):
    t = np.zeros((TROWS, ROWF), np.float32)
    t[1:NPAD + 1] = np.concatenate(slabs, axis=0)
    t[N + 1:] = 0.0            # garbage nodes >= N and tail junk rows
    return t


def _adst_stream(table, dstmap):
    """per-edge a_dst values [128, BLOCKS*K*4] f32 in (p, k, h) layout."""
    B, K, _ = dstmap.shape
    d = dstmap                                   # [B, K, 128]
    val = np.zeros((B, K, 128, 4), np.float32)
    ok = d >= 0
    val[ok] = table[d[ok] + 1][:, 70:74]
    return np.ascontiguousarray(
        val.transpose(2, 0, 1, 3).reshape(128, B * K * 4))


# --------------------------------------------------------------------------
# device kernels
# --------------------------------------------------------------------------
_KER_CACHE = {}


def _get_kernels(meta):
    key = (meta["K"], tuple(meta["calls"]))
    if key not in _KER_CACHE:
        _KER_CACHE[key] = _build_kernels(meta)
    return _KER_CACHE[key]


def _build_kernels(meta):
    import concourse.mybir as mybir
    import concourse.tile as tile
    from concourse import bacc

    K, calls = meta["K"], meta["calls"]
    NIDX16 = sum(n * 8 for _, n in calls) * BLOCKS
    dt = mybir.dt
    AF = mybir.ActivationFunctionType

    def new_nc():
        return bacc.Bacc("TRN2", target_bir_lowering=False, debug=False,
                         num_swdge_queues=4)

    # ---- L0: dense only -------------------------------------------------
    nc0 = new_nc()
    xT = nc0.declare_dram_parameter("xT", [128, NPC], dt.float32, False)
    w0 = nc0.declare_dram_parameter("wext", [128, 136], dt.float32, False)
    slab0 = nc0.declare_dram_parameter("slab", [NPC, ROWF], dt.float32, True)
    with tile.TileContext(nc0) as tc:
        with tc.tile_pool(name="p", bufs=2) as pool, \
             tc.tile_pool(name="ps", bufs=2, space="PSUM") as pps:
            xts = pool.tile([128, NPC], dt.float32, tag="xt")
            nc0.sync.dma_start(out=xts[:], in_=xT[:])
            ws = pool.tile([128, 136], dt.float32, tag="w")
            nc0.sync.dma_start(out=ws[:], in_=w0[:])
            for b in range(BLOCKS):
                ps = pps.tile([128, 136], dt.float32, tag="h")
                nc0.tensor.matmul(ps[:], lhsT=xts[:, b * 128:(b + 1) * 128],
                                  rhs=ws[:], start=True, stop=True)
                row = pool.tile([128, ROWF], dt.float32, tag="row")
                rb = row[:].bitcast(dt.bfloat16)
                nc0.scalar.activation(rb[:, 0:128], ps[:, 0:128], AF.Copy)
                nc0.vector.memset(rb[:, 128:132], 1.0)
                nc0.vector.tensor_copy(row[:, 66:74], ps[:, 128:136])
                nc0.vector.memset(row[:, 74:128], 0.0)
                nc0.sync.dma_start(out=slab0[b * 128:(b + 1) * 128, :],
                                   in_=row[:])
    _split_multiwait_ctrl(nc0)
    nc0.compile()

    # ---- edge phase (+ optional fused next dense) -----------------------
    def build_edge(last):
        nc = new_nc()
        table = nc.declare_dram_parameter("table", [TROWS, ROWF], dt.float32, False)
        gidx = nc.declare_dram_parameter("gidx", [128, NIDX16], dt.int16, False)
        dstrel = nc.declare_dram_parameter("dstrel", [128, BLOCKS * K], dt.float32, False)
        adste = nc.declare_dram_parameter("adste", [128, BLOCKS * K * 4], dt.float32, False)
        iota = nc.declare_dram_parameter("iota", [128, 128], dt.float32, False)
        NH = 1 if last else 4
        HC = 64 if last else 128
        MC = HC + NH
        C = HC // NH
        bias = nc.declare_dram_parameter("bias", [128, HC], dt.float32, False)
        if last:
            out = nc.declare_dram_parameter("out", [NPC, HC], dt.float32, True)
        else:
            ident = nc.declare_dram_parameter("ident", [128, 128], dt.float32, False)
            wnext = nc.declare_dram_parameter("wext", [128, 136], dt.float32, False)
            out = nc.declare_dram_parameter("slab", [NPC, ROWF], dt.float32, True)

        with tile.TileContext(nc) as tc:
            with tc.tile_pool(name="c", bufs=1) as cpool, \
                 tc.tile_pool(name="g", bufs=4) as gpool, \
                 tc.tile_pool(name="w", bufs=4) as wpool, \
                 tc.tile_pool(name="ps", bufs=2, space="PSUM") as pps, \
                 tc.tile_pool(name="ps2", bufs=2, space="PSUM") as pps2:
                regs = {}
                for _, nch in calls:
                    if nch * 128 not in regs:
                        regs[nch * 128] = nc.gpsimd.to_reg(nch * 128)
                iot = cpool.tile([128, 128], dt.float32, tag="iota")
                nc.sync.dma_start(out=iot[:], in_=iota[:])
                bia = cpool.tile([128, HC], dt.float32, tag="bias")
                nc.sync.dma_start(out=bia[:], in_=bias[:])
                idxs = cpool.tile([128, NIDX16], dt.int16, tag="gidx")
                nc.sync.dma_start(out=idxs[:], in_=gidx[:])
                drel = cpool.tile([128, BLOCKS * K], dt.float32, tag="drel")
                nc.sync.dma_start(out=drel[:], in_=dstrel[:])
                adst = cpool.tile([128, BLOCKS * K * 4], dt.float32, tag="adst")
                nc.sync.dma_start(out=adst[:], in_=adste[:])
                if not last:
                    idn = cpool.tile([128, 128], dt.float32, tag="ident")
                    nc.sync.dma_start(out=idn[:], in_=ident[:])
                    wnx = cpool.tile([128, 136], dt.float32, tag="wext")
                    nc.sync.dma_start(out=wnx[:], in_=wnext[:])

                tab_lo = table[0:LO_LIM, :]
                tab_hi = table[HI_OFF:TROWS, :]
                ioff = 0
                qn = 0
                for b in range(BLOCKS):
                    G = gpool.tile([128, K, ROWF], dt.float32, tag="G")
                    k0 = 0
                    for hf, nch in calls:
                        ni = nch * 128
                        nc.gpsimd.dma_gather(
                            G[:, k0:k0 + nch, :],
                            tab_lo if hf == 0 else tab_hi,
                            idxs[:, ioff:ioff + ni // 16],
                            num_idxs=ni, num_idxs_reg=regs[ni],
                            elem_size=ROWF, queue_num=qn)
                        qn = (qn + 1) % 4
                        ioff += ni // 16
                        k0 += nch
                    Gb = G[:].bitcast(dt.bfloat16)   # [128, K, 256]

                    # w = exp(lrelu(a_src + a_dst))
                    wv = wpool.tile([128, K * NH], dt.float32, tag="wv")
                    nc.vector.tensor_add(
                        wv[:].rearrange("p (k h) -> p k h", h=NH),
                        G[:, :, 66:66 + NH],
                        adst[:, b * K * 4:(b + 1) * K * 4]
                            .rearrange("p (k h) -> p k h", h=4)[:, :, 0:NH])
                    nc.scalar.activation(wv[:], wv[:], AF.Prelu, alpha=NEG)
                    nc.scalar.activation(wv[:], wv[:], AF.Exp)
                    wb = wpool.tile([128, K * NH], dt.bfloat16, tag="wb")
                    nc.scalar.activation(wb[:], wv[:], AF.Copy)

                    # one-hot(dst_rel) [128, K, 128] bf16
                    oh = wpool.tile([128, K * 128], dt.bfloat16, tag="oh")
                    nc.vector.tensor_tensor(
                        oh[:].rearrange("p (k j) -> p k j", j=128),
                        drel[:, b * K:(b + 1) * K]
                            .rearrange("p (k o) -> p k o", o=1)
                            .to_broadcast([128, K, 128]),
                        iot[:].rearrange("p (o j) -> p o j", o=1)
                            .to_broadcast([128, K, 128]),
                        op=mybir.AluOpType.is_equal)

                    # M = [h*w | w] bf16
                    M = wpool.tile([128, K * MC], dt.bfloat16, tag="M")
                    Mv = M[:].rearrange("p (k m) -> p k m", m=MC)
                    nc.vector.tensor_mul(
                        Mv[:, :, 0:HC].rearrange("p k (h c) -> p k h c", c=C),
                        Gb[:, :, 0:HC].rearrange("p k (h c) -> p k h c", c=C),
                        wb[:].rearrange("p (k h o) -> p k h o", h=NH, o=1)
                            .to_broadcast([128, K, NH, C]))
                    nc.vector.tensor_mul(
                        Mv[:, :, HC:MC],
                        Gb[:, :, 128:128 + NH],
                        wb[:].rearrange("p (k h) -> p k h", h=NH))

                    T = pps.tile([128, MC], dt.float32, tag="T")
                    for k in range(K):
                        nc.tensor.matmul(T[:],
                                         lhsT=oh[:, k * 128:(k + 1) * 128],
                                         rhs=Mv[:, k, :],
                                         start=(k == 0), stop=(k == K - 1))

                    rcp = wpool.tile([128, NH], dt.float32, tag="rcp")
                    nc.vector.reciprocal(rcp[:], T[:, HC:MC])
                    xp = wpool.tile([128, HC], dt.float32, tag="xp")
                    nc.vector.tensor_mul(
                        xp[:].rearrange("p (h c) -> p h c", c=C),
                        T[:, 0:HC].rearrange("p (h c) -> p h c", c=C),
                        rcp[:].rearrange("p (h o) -> p h o", o=1)
                            .to_broadcast([128, NH, C]))
                    nc.vector.tensor_add(xp[:], xp[:], bia[:])
                    nc.scalar.activation(xp[:], xp[:], AF.Prelu, alpha=NEG)
                    if last:
                        nc.sync.dma_start(out=out[b * 128:(b + 1) * 128, :],
                                          in_=xp[:])
                    else:
                        pt = pps2.tile([128, 128], dt.float32, tag="xt")
                        nc.tensor.transpose(out=pt[:], in_=xp[:],
                                            identity=idn[:])
                        xt = wpool.tile([128, 128], dt.float32, tag="xts")
                        nc.scalar.activation(xt[:], pt[:], AF.Copy)
                        ph = pps2.tile([128, 136], dt.float32, tag="h2")
                        nc.tensor.matmul(ph[:], lhsT=xt[:], rhs=wnx[:],
                                         start=True, stop=True)
                        row = wpool.tile([128, ROWF], dt.float32, tag="row")
                        rb = row[:].bitcast(dt.bfloat16)
                        nc.scalar.activation(rb[:, 0:128], ph[:, 0:128], AF.Copy)
                        nc.vector.memset(rb[:, 128:132], 1.0)
                        nc.vector.tensor_copy(row[:, 66:74], ph[:, 128:136])
                        nc.vector.memset(row[:, 74:128], 0.0)
                        nc.sync.dma_start(out=out[b * 128:(b + 1) * 128, :],
                                          in_=row[:])
        _split_multiwait_ctrl(nc)
        nc.compile()
        return nc

    return nc0, build_edge(False), build_edge(True)


# --------------------------------------------------------------------------
# entry point
# --------------------------------------------------------------------------
def kernel(x, edge_index, W0, as0, ad0, b0, W1, as1, ad1, b1, W2, as2, ad2, b2):
    _install_ntff_hook()
    from concourse.bass_utils import run_bass_kernel_spmd

    x = np.asarray(x, np.float32)
    meta = _prep_graph(np.asarray(edge_index))
    nc0, nc12, nc3 = _get_kernels(meta)
    cores = list(range(NCORES))
    trace = bool(os.environ.get("BASS_TRACE"))

    iota = np.tile(np.arange(128, dtype=np.float32), (128, 1))
    ident = np.eye(128, dtype=np.float32)
    w0e, w1e, w2e = _wext(W0, as0, ad0), _wext(W1, as1, ad1), _wext(W2, as2, ad2)

    total_ns = [0]

    def run(nc, maps):
        last = None
        for attempt in range(3):
            try:
                r = run_bass_kernel_spmd(nc, maps, core_ids=cores, trace=trace)
                if r.exec_time_ns:
                    total_ns[0] += int(r.exec_time_ns)
                    if os.environ.get("KERNEL_VERBOSE"):
                        print(f"[launch] exec={r.exec_time_ns}ns", file=sys.stderr)
                return r.results
            except Exception as e:  # intermittent NRT exec-unit crashes
                last = e
        raise last

    xT = np.zeros((128, NPAD), np.float32)
    xT[:, :N] = x.T
    res = run(nc0, [{"xT": np.ascontiguousarray(xT[:, c * NPC:(c + 1) * NPC]),
                     "wext": w0e} for c in cores])
    table = _assemble_table([res[c]["slab"] for c in cores])

    def edge_maps(tab, wnext, bias_vec, hc):
        bias = np.tile(np.asarray(bias_vec, np.float32)[:hc], (128, 1))
        maps = []
        for c in cores:
            pc = meta["per_core"][c]
            m = {"table": tab, "gidx": pc["gidx"], "dstrel": pc["dstrel"],
                 "adste": _adst_stream(tab, pc["dstmap"]),
                 "iota": iota, "bias": bias}
            if wnext is not None:
                m["ident"] = ident
                m["wext"] = wnext
            maps.append(m)
        return maps

    res = run(nc12, edge_maps(table, w1e, b0, 128))
    table = _assemble_table([res[c]["slab"] for c in cores])
    res = run(nc12, edge_maps(table, w2e, b1, 128))
    table = _assemble_table([res[c]["slab"] for c in cores])
    res = run(nc3, edge_maps(table, None, b2, 64))
    out = np.concatenate([res[c]["out"] for c in cores], axis=0)[:N]
    kernel.last_exec_ns = total_ns[0]
    return np.ascontiguousarray(out, dtype=np.float32)

